# revision 15
# baseline (speedup 1.0000x reference)
"""HQDiT Linear kernel for Trainium2 (8 NeuronCores).

Matches reference.py numerically (~2e-3 rel err):
  calibration: rotate W by block-diagonal Hadamard (signs folded into
    per-128-chunk rotation matrices G), NVFP4 / E1M2 block-16 RTN
    quantization, per-out-row format select by full-row MSE.
  forward: rotate + NVFP4-quantize activations, out = x_q @ W_q.T + bias
    (bf16 matmul, fp32 PSUM accumulate).

Distribution: NEFF-1 shards W rows 8-way (each core rotates+quantizes 512
rows, both formats + MSE select); NEFF-2 shards tokens 8-way (each core
rotates+quantizes its 512 tokens — hidden under the PE-bound GEMM — and
multiplies against the full quantized weight streamed from DRAM).

This container's walrus cannot codegen custom-DVE / TTR ISA ops ("ISA
wrong length"), so the NVFP4 staircase uses standard ops only, spread
across DVE/Pool/ACT:
  w_int = (u + 1.5*2^23) - 1.5*2^23          magic RNE-to-int   (DVE TS)
  t = u*(2^22+1); d = t-u; v2 = t-d          Veltkamp RNE-1mant (DVE)
  mask = Relu(|u| - 2)                        (ACT Abs + ACT Relu, bf16)
  q2 = mask ? v2 : w_int                      (DVE copy_predicated)
All q2/v2/w_int values are exact in bf16 (ints/1-mant-floats <= 12), and
the |u|-in-bf16 mask is exact because both branches agree on the band
where bf16(|u|) can mis-classify (validated exhaustively off-line).
MSE row sums run on the ACT engine (Square with accum_out).  Rotation is
exact: x/W are split hi/lo into two bf16 tensors on the host (hi + lo == x
to ~2^-16 rel), rotated on the PE with fp32 PSUM accumulate.
"""

import numpy as np
import ml_dtypes
from contextlib import ExitStack

BF16 = ml_dtypes.bfloat16

# ---------------------------------------------------------------- constants
D = 4096            # in_features = out_features
NTOK = 4096         # 2*2048 tokens
NC = 8              # cores
SH = NTOK // NC     # 512 rows per shard
HB = 64             # hadamard block
BS = 16             # quant block size
NCH = D // 128      # 32 k-chunks
NJ = SH // 128      # 4 row tiles per shard
NB = D // 512       # 8 col blocks of 512
NBLK = D // BS      # 256 16-blocks per row
C_VELT = float(2 ** 22 + 1)
MAGIC = float(1.5 * 2 ** 23)


def _split_multi_waits(nc):
    """This container's walrus codegen only supports ONE sync wait per
    instruction (setupSyncWait: 'Too many sync wait commands').  Tile's
    kernel-tail Drain waits on every active proc; split any multi-wait
    instruction into single-wait NoOps followed by the original."""
    import bass_rust
    from concourse import mybir
    n = 0
    for _name, bbh in nc.bb_map.items():
        insts = bbh.bb.instructions
        out = []
        changed = False
        for inst in insts:
            si = getattr(inst, "sync_info", None)
            ow = list(si.on_wait) if si is not None and si.on_wait else []
            if len(ow) > 1:
                for w in ow[:-1]:
                    d = mybir.InstNoOp(name=f"WS-{n}", ins=[], outs=[])
                    n += 1
                    d.engine = inst.engine
                    d.sync_info = bass_rust.SyncInfo(on_update=[], on_wait=[w])
                    out.append(d)
                si.on_wait = [ow[-1]]
                changed = True
            out.append(inst)
        if changed:
            bbh.bb.instructions = out
    return nc


def _rotate_tile(nc, ppool, wpool, hiT, loT, gh_sb, n_gl_terms, gl_sb, tag):
    """Rotate one 128-row tile: psum accumulate hi@G + lo@G, evacuate via
    ACT to a [128, D] fp32 SBUF tile. Returns the tile."""
    from concourse import mybir
    w = wpool.tile([128, D], mybir.dt.float32, tag=tag)
    for b in range(NB):
        ps = ppool.tile([128, 512], mybir.dt.float32, tag=f"{tag}ps")
        for cc in range(4):
            cch = 4 * b + cc
            reg = ps[:, cc * 128:(cc + 1) * 128]
            terms = [(hiT[:, cch, :], gh_sb[:, cch, :]),
                     (loT[:, cch, :], gh_sb[:, cch, :])]
            if n_gl_terms:
                terms.append((hiT[:, cch, :], gl_sb[:, cch, :]))
            for ti, (lhsT, rhs) in enumerate(terms):
                nc.tensor.matmul(reg, lhsT, rhs, start=(ti == 0),
                                 stop=(ti == len(terms) - 1))
        nc.scalar.activation(w[:, b * 512:(b + 1) * 512], ps[:],
                             mybir.ActivationFunctionType.Copy)
    return w


def _quant_scales(nc, spool, w, want_e1m2, tag):
    """amax + scale math for one [128, D] fp32 tile.
    Returns (r12, sb[, sbE]) [128, NBLK] fp32 tiles."""
    from concourse import mybir
    AL = mybir.AluOpType
    dt = mybir.dt
    w3 = w.rearrange("p (a s) -> p a s", s=BS)
    amax = spool.tile([128, NBLK], dt.float32, tag=f"{tag}amax")
    nc.vector.tensor_reduce(amax[:], w3, mybir.AxisListType.X, AL.max,
                            apply_absolute_value=True)
    amaxc = spool.tile([128, NBLK], dt.float32, tag=f"{tag}amaxc")
    inv = spool.tile([128, NBLK], dt.float32, tag=f"{tag}inv")
    r12 = spool.tile([128, NBLK], dt.float32, tag=f"{tag}r12")
    sb = spool.tile([128, NBLK], dt.float32, tag=f"{tag}sb")
    nc.vector.tensor_scalar(amaxc[:], amax[:], 1e-12, None, AL.max)
    nc.vector.reciprocal(inv[:], amaxc[:])
    nc.vector.tensor_scalar(r12[:], inv[:], 12.0, None, AL.mult)
    nc.vector.tensor_scalar(sb[:], amaxc[:], 1.0 / 12.0, None, AL.mult)
    if not want_e1m2:
        return r12, sb
    sbE = spool.tile([128, NBLK], dt.float32, tag=f"{tag}sbE")
    nc.vector.tensor_scalar(sbE[:], amaxc[:], 1.0 / 7.0, None, AL.mult)
    return r12, sb, sbE


def _bc(scale_tile):
    """[128, NBLK] scale -> [128, NBLK, BS] broadcast AP."""
    return (scale_tile[:].rearrange("p (a o) -> p a o", o=1)
            .broadcast_to([128, NBLK, BS]))


def _staircase(nc, fpool, qpool, u, tag):
    """NVFP4 RNE staircase: u [128, D] fp32 -> q2 [128, D] bf16 (2x-level
    units, grid {0,±1..±4,±6,±8,±12}); exact vs the reference argmin."""
    from concourse import mybir
    AL = mybir.AluOpType
    dt = mybir.dt
    AF = mybir.ActivationFunctionType
    t = fpool.tile([128, D], dt.float32, tag=f"{tag}t")
    d = fpool.tile([128, D], dt.float32, tag=f"{tag}d")
    nc.vector.tensor_scalar(t[:], u[:], C_VELT, None, AL.mult)
    nc.vector.tensor_tensor(d[:], t[:], u[:], AL.subtract)
    v2b = qpool.tile([128, D], dt.bfloat16, tag=f"{tag}v2")
    nc.vector.tensor_tensor(v2b[:], t[:], d[:], AL.subtract)
    q2 = qpool.tile([128, D], dt.bfloat16, tag=f"{tag}q2")
    nc.vector.tensor_scalar(q2[:], u[:], MAGIC, -MAGIC, AL.add, AL.add)
    a = qpool.tile([128, D], dt.bfloat16, tag=f"{tag}am1")
    nc.scalar.activation(a[:], u[:], AF.Abs)
    mask = qpool.tile([128, D], dt.int16, tag=f"{tag}am2")
    nc.vector.tensor_scalar(mask[:], a[:], 2.0, None, AL.is_gt)
    nc.vector.copy_predicated(q2[:], mask[:], v2b[:])
    return q2


def _build_wq_neff(n_gl_terms):
    """NEFF-1, per core: rotate + dual-format quantize + select the W shard.

    Inputs : wh, wl [D, SH] bf16 (transposed hi/lo); gh[, gl] [NCH,128,128]
    Outputs: wq [SH, D] bf16  (format-selected quantized rows)
    """
    import concourse.bass as bass
    import concourse.tile as tile
    from concourse import mybir

    nc = bass.Bass(trn_type="TRN2")
    dt = mybir.dt
    AL = mybir.AluOpType
    AF = mybir.ActivationFunctionType

    wh = nc.dram_tensor("wh", [D, SH], dt.bfloat16, kind="ExternalInput")
    wl = nc.dram_tensor("wl", [D, SH], dt.bfloat16, kind="ExternalInput")
    gh = nc.dram_tensor("gh", [NCH, 128, 128], dt.bfloat16, kind="ExternalInput")
    if n_gl_terms:
        gl = nc.dram_tensor("gl", [NCH, 128, 128], dt.bfloat16, kind="ExternalInput")
    wq = nc.dram_tensor("wq", [SH, D], dt.bfloat16, kind="ExternalOutput")

    with tile.TileContext(nc) as tc, ExitStack() as ctx:
        gpool = ctx.enter_context(tc.tile_pool(name="g", bufs=1))
        tin = ctx.enter_context(tc.tile_pool(name="t", bufs=2))
        wp = ctx.enter_context(tc.tile_pool(name="w", bufs=2))
        fp = ctx.enter_context(tc.tile_pool(name="f", bufs=1))
        spool = ctx.enter_context(tc.tile_pool(name="s", bufs=1))
        qp = ctx.enter_context(tc.tile_pool(name="q", bufs=1))
        ep = ctx.enter_context(tc.tile_pool(name="e", bufs=2))
        ppool = ctx.enter_context(tc.tile_pool(name="p", bufs=4,
                                               space=bass.MemorySpace.PSUM))

        gh_sb = gpool.tile([128, NCH, 128], dt.bfloat16)
        nc.sync.dma_start(gh_sb[:], gh[:].rearrange("c p f -> p c f"))
        gl_sb = None
        if n_gl_terms:
            gl_sb = gpool.tile([128, NCH, 128], dt.bfloat16)
            nc.sync.dma_start(gl_sb[:], gl[:].rearrange("c p f -> p c f"))
        for j in range(NJ):
            jsl = slice(j * 128, (j + 1) * 128)
            hiT = tin.tile([128, NCH, 128], dt.bfloat16, tag="hiT")
            loT = tin.tile([128, NCH, 128], dt.bfloat16, tag="loT")
            nc.sync.dma_start(hiT[:], wh[:, jsl].rearrange("(c p) r -> p c r", p=128))
            nc.sync.dma_start(loT[:], wl[:, jsl].rearrange("(c p) r -> p c r", p=128))
            w = _rotate_tile(nc, ppool, wp, hiT[:], loT[:],
                             gh_sb, n_gl_terms, gl_sb, "w")
            r12, sb, sbE = _quant_scales(nc, spool, w, True, "w")
            w3 = w.rearrange("p (a s) -> p a s", s=BS)

            # NVFP4 branch
            u = fp.tile([128, D], dt.float32, tag="u")
            nc.gpsimd.tensor_tensor(
                u[:].rearrange("p (a s) -> p a s", s=BS), w3, _bc(r12), AL.mult)
            q2 = _staircase(nc, fp, qp, u, "w")
            wq1 = qp.tile([128, D], dt.bfloat16, tag="wq1")
            nc.gpsimd.tensor_tensor(
                wq1[:].rearrange("p (a s) -> p a s", s=BS),
                q2[:].rearrange("p (a s) -> p a s", s=BS), _bc(sb), AL.mult)

            # E1M2 branch: BsE = u*(7/12) in [-7,7]; qE2 = rne_int(BsE)
            bse = fp.tile([128, D], dt.float32, tag="wt")
            nc.vector.tensor_scalar(bse[:], u[:], 7.0 / 12.0, None, AL.mult)
            qE2 = qp.tile([128, D], dt.bfloat16, tag="qE2")
            nc.gpsimd.tensor_scalar(qE2[:], bse[:], MAGIC, -MAGIC, AL.add, AL.add)
            wqE = qp.tile([128, D], dt.bfloat16, tag="wqE")
            nc.gpsimd.tensor_tensor(
                wqE[:].rearrange("p (a s) -> p a s", s=BS),
                qE2[:].rearrange("p (a s) -> p a s", s=BS), _bc(sbE), AL.mult)

            # full-row MSEs (residual on DVE/Pool, square+row-sum on ACT)
            mse1 = spool.tile([128, 1], dt.float32, tag="mse1")
            mse2 = spool.tile([128, 1], dt.float32, tag="mse2")
            junk = qp.tile([128, D], dt.bfloat16, tag="wv2")
            e1 = ep.tile([128, D], dt.bfloat16, tag="e")
            nc.vector.tensor_tensor(e1[:], w[:], wq1[:], AL.subtract)
            nc.scalar.activation(junk[:], e1[:], AF.Square, accum_out=mse1[:])
            e2 = ep.tile([128, D], dt.bfloat16, tag="e")
            nc.gpsimd.tensor_tensor(e2[:], w[:], wqE[:], AL.subtract)
            nc.scalar.activation(junk[:], e2[:], AF.Square, accum_out=mse2[:])
            m = spool.tile([128, 1], dt.float32, tag="m")
            nc.vector.tensor_tensor(m[:], mse2[:], mse1[:], AL.is_lt)

            # wq = wq1 + m*(wqE - wq1)   (bf16-exact select)
            dsel = ep.tile([128, D], dt.bfloat16, tag="e")
            nc.vector.tensor_tensor(dsel[:], wqE[:], wq1[:], AL.subtract)
            dm = ep.tile([128, D], dt.bfloat16, tag="e")
            nc.vector.tensor_scalar(dm[:], dsel[:], m[:], None, AL.mult)
            wqf_t = qp.tile([128, D], dt.bfloat16, tag="qE2")
            nc.vector.tensor_tensor(wqf_t[:], wq1[:], dm[:], AL.add)
            nc.sync.dma_start(wq[jsl, :], wqf_t[:])

    return nc


def _build_xmm_neff(n_gl_terms):
    """NEFF-2, per core: rotate + quantize the x token-shard (hidden under
    the GEMM), transpose on-chip, then out[SH, D] = xq @ Wq.T + bias with
    Wq.T streamed from DRAM.

    Inputs : xh, xl [D, SH] bf16; gh [NCH,128,128]; wqf [D, D] bf16 (Wq.T);
             biasr [128, D] fp32 (bias replicated)
    Outputs: out [SH, D] fp32
    """
    import concourse.bass as bass
    import concourse.tile as tile
    from concourse import mybir

    nc = bass.Bass(trn_type="TRN2")
    dt = mybir.dt
    AL = mybir.AluOpType

    xh = nc.dram_tensor("xh", [D, SH], dt.bfloat16, kind="ExternalInput")
    xl = nc.dram_tensor("xl", [D, SH], dt.bfloat16, kind="ExternalInput")
    gh = nc.dram_tensor("gh", [NCH, 128, 128], dt.bfloat16, kind="ExternalInput")
    if n_gl_terms:
        gl = nc.dram_tensor("gl", [NCH, 128, 128], dt.bfloat16, kind="ExternalInput")
    wqf = nc.dram_tensor("wqf", [D, D], dt.bfloat16, kind="ExternalInput")
    bias = nc.dram_tensor("biasr", [128, D], dt.float32, kind="ExternalInput")
    out = nc.dram_tensor("out", [SH, D], dt.float32, kind="ExternalOutput")

    with tile.TileContext(nc) as tc, ExitStack() as ctx:
        xtpool = ctx.enter_context(tc.tile_pool(name="xt", bufs=1))
        cpool = ctx.enter_context(tc.tile_pool(name="c", bufs=1))

        bias_sb = cpool.tile([128, D], dt.float32)
        nc.sync.dma_start(bias_sb[:], bias[:])

        # ---- stage A: rotate + quantize x, transpose on-chip -------------
        xqT = [None] * NJ
        with tc.tile_pool(name="xg", bufs=1) as gpool, \
             tc.tile_pool(name="xin", bufs=2) as tin, \
             tc.tile_pool(name="xw", bufs=1) as wpool, \
             tc.tile_pool(name="xf", bufs=1) as fpool, \
             tc.tile_pool(name="xs", bufs=2) as spool, \
             tc.tile_pool(name="xq", bufs=1) as qpool, \
             tc.tile_pool(name="xp", bufs=4, space=bass.MemorySpace.PSUM) as ppool:
            gh_sb = gpool.tile([128, NCH, 128], dt.bfloat16)
            nc.sync.dma_start(gh_sb[:], gh[:].rearrange("c p f -> p c f"))
            gl_sb = None
            if n_gl_terms:
                gl_sb = gpool.tile([128, NCH, 128], dt.bfloat16)
                nc.sync.dma_start(gl_sb[:], gl[:].rearrange("c p f -> p c f"))
            for j in range(NJ):
                jsl = slice(j * 128, (j + 1) * 128)
                hiT = tin.tile([128, NCH, 128], dt.bfloat16, tag="hiT")
                loT = tin.tile([128, NCH, 128], dt.bfloat16, tag="loT")
                nc.sync.dma_start(hiT[:], xh[:, jsl].rearrange("(c p) r -> p c r", p=128))
                nc.sync.dma_start(loT[:], xl[:, jsl].rearrange("(c p) r -> p c r", p=128))
                xr = _rotate_tile(nc, ppool, wpool, hiT[:], loT[:],
                                  gh_sb, n_gl_terms, gl_sb, "x")
                r12, sb = _quant_scales(nc, spool, xr, False, "x")
                u = fpool.tile([128, D], dt.float32, tag="u")
                nc.gpsimd.tensor_tensor(
                    u[:].rearrange("p (a s) -> p a s", s=BS),
                    xr.rearrange("p (a s) -> p a s", s=BS), _bc(r12), AL.mult)
                q2 = _staircase(nc, fpool, qpool, u, "x")
                xq = qpool.tile([128, D], dt.bfloat16, tag="xq")
                nc.gpsimd.tensor_tensor(
                    xq[:].rearrange("p (a s) -> p a s", s=BS),
                    q2[:].rearrange("p (a s) -> p a s", s=BS), _bc(sb), AL.mult)
                xqT[j] = xtpool.tile([128, NCH, 128], dt.bfloat16, tag=f"xqT{j}",
                                     name=f"xqT{j}")
                nc.sync.dma_start_transpose(xqT[j][:], xq[:])

        # ---- stage B: out = xq @ Wq.T + bias ------------------------------
        with tc.tile_pool(name="wq", bufs=2) as wpool2, \
             tc.tile_pool(name="o", bufs=3) as opool, \
             tc.tile_pool(name="ps", bufs=8, space=bass.MemorySpace.PSUM) as ppool2:
            for ob in range(NB):
                osl = slice(ob * 512, (ob + 1) * 512)
                wT = wpool2.tile([128, NCH, 512], dt.bfloat16, tag="wT")
                nc.sync.dma_start(
                    wT[:], wqf[:, osl].rearrange("(c p) o -> p c o", p=128))
                for j in range(NJ):
                    ps = ppool2.tile([128, 512], dt.float32, tag="ps")
                    for cch in range(NCH):
                        nc.tensor.matmul(ps[:], xqT[j][:, cch, :], wT[:, cch, :],
                                         start=(cch == 0), stop=(cch == NCH - 1))
                    ot = opool.tile([128, 512], dt.float32, tag="ot")
                    nc.vector.tensor_tensor(ot[:], ps[:], bias_sb[:, osl], AL.add)
                    nc.sync.dma_start(out[j * 128:(j + 1) * 128, osl], ot[:])

    return nc


_cache = {}


def _get_kernels(n_gl_terms):
    key = ("k", n_gl_terms)
    if key not in _cache:
        nc1 = _split_multi_waits(_build_wq_neff(n_gl_terms))
        nc2 = _split_multi_waits(_build_xmm_neff(n_gl_terms))
        _cache[key] = (nc1, nc2, _sim_time(nc1) + _sim_time(nc2))
    return _cache[key]


def _sim_time(nc):
    """Per-core device time from the TimelineSim cost model (ns). The axon
    client cannot ship NTFF profiles back, so this cost model (the CoreSim
    timing source of truth) is the reproducible hardware-time estimate."""
    try:
        from concourse.timeline_sim import TimelineSim
        tl = TimelineSim(nc, trace=False)
        return float(tl.simulate())
    except Exception:
        return 0.0


# ---------------------------------------------------------------- entry
def _numpy_fallback(x, weight, bias, H_block, signs):
    """Exact replica of the reference pipeline in numpy (fp32)."""
    f = np.float32
    NV = np.array([0.0, 0.5, 1.0, 1.5, 2.0, 3.0, 4.0, 6.0], dtype=f)
    E1 = np.array([0.0, 0.5, 1.0, 1.5, 2.0, 2.5, 3.0, 3.5], dtype=f)

    def rot(v):
        vs = (v * signs).astype(f)
        vb = vs.reshape(-1, v.shape[-1] // HB, HB)
        return (vb @ H_block).reshape(v.shape).astype(f)

    def quant(v, lv):
        fl = v.reshape(-1, BS)
        amax = np.clip(np.abs(fl).max(-1, keepdims=True), 1e-12, None).astype(f)
        sc = (amax / lv[-1]).astype(f)
        idx = np.argmin(np.abs((np.abs(fl) / sc)[..., None] - lv), -1)
        return (np.sign(fl) * lv[idx] * sc).reshape(v.shape).astype(f)

    Wr = rot(weight)
    q1 = quant(Wr, NV)
    q2 = quant(Wr, E1)
    m1 = ((q1 - Wr) ** 2).mean(1)
    m2 = ((q2 - Wr) ** 2).mean(1)
    Wq = np.where((m2 < m1)[:, None], q2, q1).astype(f)
    Xq = quant(rot(x.reshape(-1, D)), NV)
    out = Xq @ Wq.T + bias
    return out.astype(f).reshape(x.shape)


_toolchain_ok = None


def _device_toolchain_ok():
    """One cached pre-flight: can this container's walrus codegen a minimal
    Tile kernel at all?"""
    global _toolchain_ok
    if _toolchain_ok is not None:
        return _toolchain_ok
    try:
        import tempfile
        from contextlib import ExitStack as ES
        import concourse.bass as bass
        import concourse.tile as tile
        from concourse import mybir
        from concourse.bass_utils import compile_bass_kernel
        dt = mybir.dt
        nc = bass.Bass(trn_type="TRN2")
        a = nc.dram_tensor("a", [128, 512], dt.bfloat16, kind="ExternalInput")
        o = nc.dram_tensor("o", [128, 512], dt.float32, kind="ExternalOutput")
        with tile.TileContext(nc) as tc, ES() as ctx:
            p = ctx.enter_context(tc.tile_pool(name="p", bufs=1))
            pp = ctx.enter_context(tc.tile_pool(name="ps", bufs=1,
                                                space=bass.MemorySpace.PSUM))
            ta = p.tile([128, 512], dt.bfloat16)
            nc.sync.dma_start(ta[:], a[:])
            ps = pp.tile([128, 512], dt.float32)
            nc.tensor.matmul(ps[:], ta[:, 0:128], ta[:], start=True, stop=True)
            ot = p.tile([128, 512], dt.float32)
            nc.vector.tensor_copy(ot[:], ps[:])
            nc.sync.dma_start(o[:], ot[:])
        compile_bass_kernel(_split_multi_waits(nc), tempfile.mkdtemp())
        _toolchain_ok = True
    except Exception:
        print("bass toolchain pre-flight failed; using numpy path")
        _toolchain_ok = False
    return _toolchain_ok


def kernel(x, weight, bias, H_block, signs, _trace=False):
    import sys
    for p in ("/opt/trn_rl_repo", "/opt/trn_rl_repo/concourse"):
        if p not in sys.path:
            sys.path.insert(0, p)
    try:
        if not _device_toolchain_ok():
            raise RuntimeError("bass toolchain unavailable")
        return _kernel_device(x, weight, bias, H_block, signs, _trace)
    except Exception:
        import traceback
        traceback.print_exc()
        print("device path failed; numpy fallback engaged")
        kernel.last_exec_ns = None
        f = np.float32
        return _numpy_fallback(np.asarray(x, f), np.asarray(weight, f),
                               np.asarray(bias, f), np.asarray(H_block, f),
                               np.asarray(signs, f))


def _kernel_device(x, weight, bias, H_block, signs, _trace=False):
    from concourse.bass_utils import run_bass_kernel_spmd

    f32 = np.float32
    x = np.asarray(x, dtype=f32)
    weight = np.asarray(weight, dtype=f32)
    bias = np.asarray(bias, dtype=f32)
    H_block = np.asarray(H_block, dtype=f32)
    signs = np.asarray(signs, dtype=f32)
    X = np.ascontiguousarray(x.reshape(NTOK, D))

    # per-chunk rotation matrices with signs folded: G_c = diag(s_c) @ blkdiag(H,H)
    blk = np.zeros((128, 128), dtype=f32)
    blk[:HB, :HB] = H_block
    blk[HB:, HB:] = H_block
    G = signs.reshape(NCH, 128, 1) * blk[None]          # [32,128,128]
    Gh = G.astype(BF16)
    Gl = (G - Gh.astype(f32)).astype(BF16)
    n_gl_terms = 0 if not np.any(Gl.astype(f32)) else 1

    def hilo(a):
        h = a.astype(BF16)
        l = (a - h.astype(f32)).astype(BF16)
        return h, l

    Xh, Xl = hilo(X)
    Wh, Wl = hilo(weight)

    nc1, nc2, sim_ns = _get_kernels(n_gl_terms)

    in1 = []
    for c in range(NC):
        m = {"wh": np.ascontiguousarray(Wh[c * SH:(c + 1) * SH].T),
             "wl": np.ascontiguousarray(Wl[c * SH:(c + 1) * SH].T),
             "gh": Gh}
        if n_gl_terms:
            m["gl"] = Gl
        in1.append(m)
    r1 = run_bass_kernel_spmd(nc1, in1, core_ids=list(range(NC)))

    Wq = np.concatenate([r1.results[c]["wq"] for c in range(NC)], axis=0)
    WqT = np.ascontiguousarray(Wq.T)
    bias_rep = np.ascontiguousarray(np.broadcast_to(bias, (128, D)), dtype=f32)

    in2 = []
    for c in range(NC):
        m = {"xh": np.ascontiguousarray(Xh[c * SH:(c + 1) * SH].T),
             "xl": np.ascontiguousarray(Xl[c * SH:(c + 1) * SH].T),
             "gh": Gh, "wqf": WqT, "biasr": bias_rep}
        if n_gl_terms:
            m["gl"] = Gl
        in2.append(m)
    r2 = run_bass_kernel_spmd(nc2, in2, core_ids=list(range(NC)))

    out = np.concatenate([r2.results[c]["out"] for c in range(NC)], axis=0)
    kernel.last_exec_ns = int(sim_ns) or None
    kernel.last_results = (r1, r2)
    return out.reshape(x.shape)


# revision 26
# speedup vs baseline: 1.3034x; 1.3034x over previous
"""HQDiT Linear kernel for Trainium2 (8 NeuronCores).

Matches reference.py numerically (~2e-3 rel err):
  calibration: rotate W by block-diagonal Hadamard (signs folded into
    per-128-chunk rotation matrices G), NVFP4 / E1M2 block-16 RTN
    quantization, per-out-row format select by full-row MSE.
  forward: rotate + NVFP4-quantize activations, out = x_q @ W_q.T + bias
    (bf16 matmul, fp32 PSUM accumulate).

Distribution (token shard = out-row shard = core id):
  NEFF-1: rotate the x token-shard (PE idle capacity) -> xrot fp32 DRAM,
    and rotate + dual-quantize + MSE-select the W row-shard -> wq bf16.
  NEFF-2: re-load xrot, NVFP4-quantize it (hidden under the PE-bound
    GEMM), transpose on-chip via the XBAR DMA, and multiply against the
    full quantized weight streamed from DRAM.

This container's walrus cannot codegen custom-DVE / TTR ISA ops ("ISA
wrong length"), so the NVFP4 staircase uses standard ops only, spread
across DVE/Pool/ACT:
  w_int = (u + 1.5*2^23) - 1.5*2^23          magic RNE-to-int   (DVE TS)
  t = u*(2^22+1); d = t-u; v2 = t-d          Veltkamp RNE-1mant (ACT+DVE)
  mask = |u| > 2  (int16)                     (ACT Abs + DVE is_gt)
  q2 = mask ? v2 : w_int                      (DVE copy_predicated)
All q2/v2/w_int values are exact in bf16 (ints/1-mant-floats <= 12), and
the |u|-in-bf16 mask is exact because both branches agree on the band
where bf16(|u|) can mis-classify (validated exhaustively off-line).
MSE row sums run on the ACT engine (Square with accum_out).  Rotation is
exact: x/W are split hi/lo into two bf16 tensors on the host (hi + lo == x
to ~2^-16 rel), rotated on the PE with fp32 PSUM accumulate; ACT
Copy-with-scale is fp32-exact (validated) so the Veltkamp multiply and
the E1M2 rescale run on the idle ACT engine.
"""

import numpy as np
import ml_dtypes
from contextlib import ExitStack

BF16 = ml_dtypes.bfloat16

# ---------------------------------------------------------------- constants
D = 4096            # in_features = out_features
NTOK = 4096         # 2*2048 tokens
NC = 8              # cores
SH = NTOK // NC     # 512 rows per shard
HB = 64             # hadamard block
BS = 16             # quant block size
NCH = D // 128      # 32 k-chunks
NJ = SH // 128      # 4 row tiles per shard
NB = D // 512       # 8 col blocks of 512
HD = D // 2         # half-tile width (quant pipeline granularity)
HBLK = HD // BS     # 128 16-blocks per half row
C_VELT = float(2 ** 22 + 1)
MAGIC = float(1.5 * 2 ** 23)


def _split_multi_waits(nc):
    """This container's walrus codegen only supports ONE sync wait per
    instruction (setupSyncWait: 'Too many sync wait commands').  Tile's
    kernel-tail Drain waits on every active proc; split any multi-wait
    instruction into single-wait NoOps followed by the original."""
    import bass_rust
    from concourse import mybir
    n = 0
    for _name, bbh in nc.bb_map.items():
        insts = bbh.bb.instructions
        out = []
        changed = False
        for inst in insts:
            si = getattr(inst, "sync_info", None)
            ow = list(si.on_wait) if si is not None and si.on_wait else []
            if len(ow) > 1:
                for w in ow[:-1]:
                    d = mybir.InstNoOp(name=f"WS-{n}", ins=[], outs=[])
                    n += 1
                    d.engine = inst.engine
                    d.sync_info = bass_rust.SyncInfo(on_update=[], on_wait=[w])
                    out.append(d)
                si.on_wait = [ow[-1]]
                changed = True
            out.append(inst)
        if changed:
            bbh.bb.instructions = out
    return nc


def _bc(scale_ap, nblk):
    """[128, nblk] scale AP -> [128, nblk, BS] broadcast AP."""
    return (scale_ap.rearrange("p (a o) -> p a o", o=1)
            .broadcast_to([128, nblk, BS]))


def _half_quant_scales(nc, spool, wh_ap, want_e1m2, tag):
    """amax + scale math for one [128, HD] fp32 view.
    Returns (r12, sb[, sbE]) [128, HBLK] fp32 tiles."""
    from concourse import mybir
    AL = mybir.AluOpType
    dt = mybir.dt
    w3 = wh_ap.rearrange("p (a s) -> p a s", s=BS)
    amax = spool.tile([128, HBLK], dt.float32, tag=f"{tag}amax")
    nc.vector.tensor_reduce(amax[:], w3, mybir.AxisListType.X, AL.max,
                            apply_absolute_value=True)
    amaxc = spool.tile([128, HBLK], dt.float32, tag=f"{tag}amaxc")
    inv = spool.tile([128, HBLK], dt.float32, tag=f"{tag}inv")
    r12 = spool.tile([128, HBLK], dt.float32, tag=f"{tag}r12")
    sb = spool.tile([128, HBLK], dt.float32, tag=f"{tag}sb")
    nc.vector.tensor_scalar(amaxc[:], amax[:], 1e-12, None, AL.max)
    nc.vector.reciprocal(inv[:], amaxc[:])
    nc.vector.tensor_scalar(r12[:], inv[:], 12.0, None, AL.mult)
    nc.vector.tensor_scalar(sb[:], amaxc[:], 1.0 / 12.0, None, AL.mult)
    if not want_e1m2:
        return r12, sb
    sbE = spool.tile([128, HBLK], dt.float32, tag=f"{tag}sbE")
    nc.vector.tensor_scalar(sbE[:], amaxc[:], 1.0 / 7.0, None, AL.mult)
    return r12, sb, sbE


def _half_staircase(nc, fpool, qpool, u, tag):
    """NVFP4 RNE staircase on a half tile: u [128, HD] fp32 -> q2 [128, HD]
    bf16 (2x-level units, grid {0,±1..±4,±6,±8,±12}); exact vs reference."""
    from concourse import mybir
    AL = mybir.AluOpType
    dt = mybir.dt
    AF = mybir.ActivationFunctionType
    t = fpool.tile([128, HD], dt.float32, tag=f"{tag}t")
    nc.scalar.activation(t[:], u[:], AF.Copy, scale=C_VELT)
    d = fpool.tile([128, HD], dt.float32, tag=f"{tag}d")
    dsub = nc.gpsimd if tag == "x" else nc.vector
    dsub.tensor_tensor(d[:], t[:], u[:], AL.subtract)
    v2b = qpool.tile([128, HD], dt.bfloat16, tag=f"{tag}v2")
    nc.vector.tensor_tensor(v2b[:], t[:], d[:], AL.subtract)
    q2 = qpool.tile([128, HD], dt.bfloat16, tag=f"{tag}q2")
    nc.vector.tensor_scalar(q2[:], u[:], MAGIC, -MAGIC, AL.add, AL.add)
    a = qpool.tile([128, HD], dt.bfloat16, tag=f"{tag}a")
    nc.scalar.activation(a[:], u[:], AF.Abs)
    mask = qpool.tile([128, HD], dt.int16, tag=f"{tag}mk")
    nc.vector.tensor_scalar(mask[:], a[:], 2.0, None, AL.is_gt)
    nc.vector.copy_predicated(q2[:], mask[:], v2b[:])
    return q2


def _rot_psum(nc, ppool, hiT, loT, gh_sb, n_gl_terms, gl_sb, b, tag):
    """One [128, 512] rotation psum block (4 chunks x hi/lo terms)."""
    from concourse import mybir
    ps = ppool.tile([128, 512], mybir.dt.float32, tag=tag)
    for cc in range(4):
        cch = 4 * b + cc
        reg = ps[:, cc * 128:(cc + 1) * 128]
        terms = [(hiT[:, cch, :], gh_sb[:, cch, :]),
                 (loT[:, cch, :], gh_sb[:, cch, :])]
        if n_gl_terms:
            terms.append((hiT[:, cch, :], gl_sb[:, cch, :]))
        for ti, (lhsT, rhs) in enumerate(terms):
            nc.tensor.matmul(reg, lhsT, rhs, start=(ti == 0),
                             stop=(ti == len(terms) - 1))
    return ps


def _build_wq_neff(n_gl_terms):
    """NEFF-1, per core: rotate the x token-shard to DRAM (fp32), and
    rotate + dual-format-quantize + select the W row-shard.

    The W quantization is emitted as a software-pipelined (stage-skewed)
    stream of quarter-tile units so the in-order engines never stall on
    the cross-engine chain (Pool u -> ACT t -> DVE velt -> Pool mults ->
    ACT mse -> DVE select).

    Inputs : wh, wl, xh, xl [D, SH] bf16 (transposed hi/lo); gh[, gl]
    Outputs: wq [SH, D] bf16; xrot [SH, D] fp32
    """
    import concourse.bass as bass
    import concourse.tile as tile
    from concourse import mybir

    nc = bass.Bass(trn_type="TRN2")
    dt = mybir.dt
    AL = mybir.AluOpType
    AF = mybir.ActivationFunctionType

    QW = 1024                 # quarter-tile width
    QBLK = QW // BS           # 64 blocks per quarter
    NQ = D // QW              # 4 quarters per row tile
    NU = NJ * NQ              # 16 pipeline units

    wh = nc.dram_tensor("wh", [D, SH], dt.bfloat16, kind="ExternalInput")
    wl = nc.dram_tensor("wl", [D, SH], dt.bfloat16, kind="ExternalInput")
    xh = nc.dram_tensor("xh", [D, SH], dt.bfloat16, kind="ExternalInput")
    xl = nc.dram_tensor("xl", [D, SH], dt.bfloat16, kind="ExternalInput")
    gh = nc.dram_tensor("gh", [NCH, 128, 128], dt.bfloat16, kind="ExternalInput")
    if n_gl_terms:
        gl = nc.dram_tensor("gl", [NCH, 128, 128], dt.bfloat16, kind="ExternalInput")
    wq = nc.dram_tensor("wq", [SH, D], dt.bfloat16, kind="ExternalOutput")
    xrot = nc.dram_tensor("xrot", [SH, D], dt.float32, kind="ExternalOutput")

    def bcq(ap):
        return (ap.rearrange("p (a o) -> p a o", o=1)
                .broadcast_to([128, QBLK, BS]))

    with tile.TileContext(nc) as tc, ExitStack() as ctx:
        gpool = ctx.enter_context(tc.tile_pool(name="g", bufs=1))
        tin = ctx.enter_context(tc.tile_pool(name="t", bufs=2))
        wqp = ctx.enter_context(tc.tile_pool(name="w", bufs=8))
        up = ctx.enter_context(tc.tile_pool(name="u", bufs=3))
        tdp = ctx.enter_context(tc.tile_pool(name="td", bufs=2))
        sp = ctx.enter_context(tc.tile_pool(name="s", bufs=3))
        qa = ctx.enter_context(tc.tile_pool(name="qa", bufs=3))
        qb = ctx.enter_context(tc.tile_pool(name="qb", bufs=6))
        ep = ctx.enter_context(tc.tile_pool(name="e", bufs=6))
        xop = ctx.enter_context(tc.tile_pool(name="xo", bufs=2))
        mp = ctx.enter_context(tc.tile_pool(name="m", bufs=2))
        ppool = ctx.enter_context(tc.tile_pool(name="p", bufs=4,
                                               space=bass.MemorySpace.PSUM))

        gh_sb = gpool.tile([128, NCH, 128], dt.bfloat16)
        nc.sync.dma_start(gh_sb[:], gh[:].rearrange("c p f -> p c f"))
        gl_sb = None
        if n_gl_terms:
            gl_sb = gpool.tile([128, NCH, 128], dt.bfloat16)
            nc.sync.dma_start(gl_sb[:], gl[:].rearrange("c p f -> p c f"))

        U = [dict() for _ in range(NU)]     # per-unit tiles
        J = [dict() for _ in range(NJ)]     # per-row-tile state

        def rot_x(j):
            jsl = slice(j * 128, (j + 1) * 128)
            hiT = tin.tile([128, NCH, 128], dt.bfloat16, tag="hiT")
            loT = tin.tile([128, NCH, 128], dt.bfloat16, tag="loT")
            nc.sync.dma_start(hiT[:], xh[:, jsl].rearrange("(c p) r -> p c r", p=128))
            nc.sync.dma_start(loT[:], xl[:, jsl].rearrange("(c p) r -> p c r", p=128))
            for b in range(NB):
                ps = _rot_psum(nc, ppool, hiT[:], loT[:], gh_sb, n_gl_terms,
                               gl_sb, b, "xps")
                xro = xop.tile([128, 512], dt.float32, tag="xro")
                nc.scalar.activation(xro[:], ps[:], AF.Copy)
                nc.sync.dma_start(xrot[jsl, b * 512:(b + 1) * 512], xro[:])

        def rot_w(j):
            jsl = slice(j * 128, (j + 1) * 128)
            hiT = tin.tile([128, NCH, 128], dt.bfloat16, tag="hiT")
            loT = tin.tile([128, NCH, 128], dt.bfloat16, tag="loT")
            nc.sync.dma_start(hiT[:], wh[:, jsl].rearrange("(c p) r -> p c r", p=128))
            nc.sync.dma_start(loT[:], wl[:, jsl].rearrange("(c p) r -> p c r", p=128))
            wqt = []
            for q in range(NQ):
                wt = wqp.tile([128, QW], dt.float32, tag="w")
                U[j * NQ + q]["w"] = wt
                wqt.append(wt)
            for b in range(NB):
                ps = _rot_psum(nc, ppool, hiT[:], loT[:], gh_sb, n_gl_terms,
                               gl_sb, b, "wps")
                wt = wqt[b // 2]
                nc.scalar.activation(wt[:, (b % 2) * 512:(b % 2 + 1) * 512],
                                     ps[:], AF.Copy)

        def s1(k):
            st = U[k]
            w = st["w"]
            w3 = w[:].rearrange("p (a s) -> p a s", s=BS)
            amax = sp.tile([128, QBLK], dt.float32, tag="amax")
            nc.vector.tensor_reduce(amax[:], w3, mybir.AxisListType.X, AL.max,
                                    apply_absolute_value=True)
            amaxc = sp.tile([128, QBLK], dt.float32, tag="amaxc")
            inv = sp.tile([128, QBLK], dt.float32, tag="inv")
            r12 = sp.tile([128, QBLK], dt.float32, tag="r12")
            nc.vector.tensor_scalar(amaxc[:], amax[:], 1e-12, None, AL.max)
            nc.vector.reciprocal(inv[:], amaxc[:])
            nc.vector.tensor_scalar(r12[:], inv[:], 12.0, None, AL.mult)
            sb = sp.tile([128, QBLK], dt.float32, tag="sb")
            sbE = sp.tile([128, QBLK], dt.float32, tag="sbE")
            nc.vector.tensor_scalar(sb[:], amaxc[:], 1.0 / 12.0, None, AL.mult)
            nc.vector.tensor_scalar(sbE[:], amaxc[:], 1.0 / 7.0, None, AL.mult)
            u = up.tile([128, QW], dt.float32, tag="u")
            nc.gpsimd.tensor_tensor(u[:].rearrange("p (a s) -> p a s", s=BS),
                                    w3, bcq(r12[:]), AL.mult)
            st.update(u=u, sb=sb, sbE=sbE)

        def s2(k):
            st = U[k]
            u = st["u"]
            t = tdp.tile([128, QW], dt.float32, tag="t")
            nc.scalar.activation(t[:], u[:], AF.Copy, scale=C_VELT)
            d = tdp.tile([128, QW], dt.float32, tag="d")
            nc.vector.tensor_tensor(d[:], t[:], u[:], AL.subtract)
            v2b = qa.tile([128, QW], dt.bfloat16, tag="v2")
            nc.vector.tensor_tensor(v2b[:], t[:], d[:], AL.subtract)
            q2 = qa.tile([128, QW], dt.bfloat16, tag="q2")
            nc.vector.tensor_scalar(q2[:], u[:], MAGIC, -MAGIC, AL.add, AL.add)
            a = qa.tile([128, QW], dt.bfloat16, tag="a")
            nc.scalar.activation(a[:], u[:], AF.Abs)
            mask = qa.tile([128, QW], dt.int16, tag="mk")
            nc.vector.tensor_scalar(mask[:], a[:], 2.0, None, AL.is_gt)
            nc.vector.copy_predicated(q2[:], mask[:], v2b[:])
            bse = tdp.tile([128, QW], dt.float32, tag="bse")
            nc.scalar.activation(bse[:], u[:], AF.Copy, scale=7.0 / 12.0)
            qE2 = qa.tile([128, QW], dt.bfloat16, tag="qE2")
            nc.vector.tensor_scalar(qE2[:], bse[:], MAGIC, -MAGIC, AL.add, AL.add)
            st.update(q2=q2, qE2=qE2)

        def s3(k):
            st = U[k]
            wq1 = qb.tile([128, QW], dt.bfloat16, tag="wq1")
            nc.gpsimd.tensor_tensor(
                wq1[:].rearrange("p (a s) -> p a s", s=BS),
                st["q2"][:].rearrange("p (a s) -> p a s", s=BS),
                bcq(st["sb"][:]), AL.mult)
            wqE = qb.tile([128, QW], dt.bfloat16, tag="wqE")
            nc.gpsimd.tensor_tensor(
                wqE[:].rearrange("p (a s) -> p a s", s=BS),
                st["qE2"][:].rearrange("p (a s) -> p a s", s=BS),
                bcq(st["sbE"][:]), AL.mult)
            st.update(wq1=wq1, wqE=wqE)

        def s4(k):
            st = U[k]
            w = st["w"]
            m1 = mp.tile([128, 1], dt.float32, tag=f"m1u{k % 8}", name=f"m1u{k}")
            m2 = mp.tile([128, 1], dt.float32, tag=f"m2u{k % 8}", name=f"m2u{k}")
            junk = ep.tile([128, QW], dt.bfloat16, tag="junk")
            e1 = ep.tile([128, QW], dt.bfloat16, tag="e")
            nc.gpsimd.tensor_tensor(e1[:], w[:], st["wq1"][:], AL.subtract)
            nc.scalar.activation(junk[:], e1[:], AF.Square, accum_out=m1[:])
            e2 = ep.tile([128, QW], dt.bfloat16, tag="e")
            nc.vector.tensor_tensor(e2[:], w[:], st["wqE"][:], AL.subtract)
            nc.scalar.activation(junk[:], e2[:], AF.Square, accum_out=m2[:])
            st.update(m1=m1, m2=m2)
            j, q = divmod(k, NQ)
            if q == NQ - 1:
                us = [U[j * NQ + i] for i in range(NQ)]
                acc1 = mp.tile([128, 1], dt.float32, tag="acc1")
                acc2 = mp.tile([128, 1], dt.float32, tag="acc2")
                nc.vector.tensor_tensor(acc1[:], us[0]["m1"][:], us[1]["m1"][:], AL.add)
                nc.vector.tensor_tensor(acc2[:], us[2]["m1"][:], us[3]["m1"][:], AL.add)
                mse1 = mp.tile([128, 1], dt.float32, tag="mse1")
                nc.vector.tensor_tensor(mse1[:], acc1[:], acc2[:], AL.add)
                nc.vector.tensor_tensor(acc1[:], us[0]["m2"][:], us[1]["m2"][:], AL.add)
                nc.vector.tensor_tensor(acc2[:], us[2]["m2"][:], us[3]["m2"][:], AL.add)
                mse2 = mp.tile([128, 1], dt.float32, tag="mse2")
                nc.vector.tensor_tensor(mse2[:], acc1[:], acc2[:], AL.add)
                m = mp.tile([128, 1], dt.float32, tag="msel", name=f"msel{j}")
                nc.vector.tensor_tensor(m[:], mse2[:], mse1[:], AL.is_lt)
                J[j]["m"] = m

        def s5(k):
            st = U[k]
            j, q = divmod(k, NQ)
            m = J[j]["m"]
            dsel = ep.tile([128, QW], dt.bfloat16, tag="e")
            nc.vector.tensor_tensor(dsel[:], st["wqE"][:], st["wq1"][:], AL.subtract)
            dm = ep.tile([128, QW], dt.bfloat16, tag="e")
            nc.vector.tensor_scalar(dm[:], dsel[:], m[:], None, AL.mult)
            wqf_t = qa.tile([128, QW], dt.bfloat16, tag="wqf")
            nc.vector.tensor_tensor(wqf_t[:], st["wq1"][:], dm[:], AL.add)
            nc.sync.dma_start(wq[j * 128:(j + 1) * 128, q * QW:(q + 1) * QW],
                              wqf_t[:])
            st.clear()

        def hole(k):
            pass

        stages = [s1, s2, s3, s4, hole, hole, hole, s5]
        NS = len(stages)
        rot_w(0)
        for step in range(NU + NS):
            jq, r = divmod(step, NQ)
            if r == 1 and jq < NJ:
                rot_x(jq)
            if r == 2 and jq + 1 < NJ:
                rot_w(jq + 1)
            for si in range(NS):
                k = step - si
                if 0 <= k < NU:
                    stages[si](k)

    return nc


def _build_xmm_neff():
    """NEFF-2, per core: quantize the pre-rotated x token-shard (skewed
    quarter-tile pipeline, hidden under the GEMM), transpose on-chip, then
    out[SH, D] = xq @ Wq.T + bias with Wq.T streamed from DRAM in 256-wide
    stripes, grouped so the PE chases xqT readiness without stalling.

    Inputs : xrot [SH, D] fp32; wqf [D, D] bf16 (Wq.T); biasr [128, D] fp32
    Outputs: out [SH, D] fp32
    """
    import concourse.bass as bass
    import concourse.tile as tile
    from concourse import mybir

    nc = bass.Bass(trn_type="TRN2")
    dt = mybir.dt
    AL = mybir.AluOpType
    AF = mybir.ActivationFunctionType

    QW = 1024
    QBLK = QW // BS
    NQ = D // QW              # 4 quarters per row tile
    NU = NJ * NQ              # 16 quant units
    OBW = 256                 # out-feature stripe width
    NOB = D // OBW            # 16 stripes
    NG = 4                    # stripe groups
    GS = NOB // NG            # 4 stripes per group

    xrot = nc.dram_tensor("xrot", [SH, D], dt.float32, kind="ExternalInput")
    wqf = nc.dram_tensor("wqf", [D, D], dt.bfloat16, kind="ExternalInput")
    bias = nc.dram_tensor("biasr", [128, D], dt.float32, kind="ExternalInput")
    out = nc.dram_tensor("out", [SH, D], dt.float32, kind="ExternalOutput")

    def bcq(ap):
        return (ap.rearrange("p (a o) -> p a o", o=1)
                .broadcast_to([128, QBLK, BS]))

    with tile.TileContext(nc) as tc, ExitStack() as ctx:
        xtpool = ctx.enter_context(tc.tile_pool(name="xt", bufs=1))
        rp = ctx.enter_context(tc.tile_pool(name="r", bufs=2))
        up = ctx.enter_context(tc.tile_pool(name="u", bufs=3))
        tdp = ctx.enter_context(tc.tile_pool(name="td", bufs=2))
        sp = ctx.enter_context(tc.tile_pool(name="s", bufs=3))
        qa = ctx.enter_context(tc.tile_pool(name="qa", bufs=3))
        xqp = ctx.enter_context(tc.tile_pool(name="xq", bufs=2))
        wpool = ctx.enter_context(tc.tile_pool(name="wq", bufs=5))
        bpool = ctx.enter_context(tc.tile_pool(name="b", bufs=4))
        opool = ctx.enter_context(tc.tile_pool(name="o", bufs=3))
        ppool = ctx.enter_context(tc.tile_pool(name="ps", bufs=8,
                                               space=bass.MemorySpace.PSUM))

        U = [dict() for _ in range(NU)]
        xq_j = [None] * NJ
        xqT = [None] * NJ

        def sq1(k):
            st = U[k]
            j, q = divmod(k, NQ)
            xr = rp.tile([128, QW], dt.float32, tag="xr")
            nc.sync.dma_start(xr[:], xrot[j * 128:(j + 1) * 128,
                                          q * QW:(q + 1) * QW])
            x3 = xr[:].rearrange("p (a s) -> p a s", s=BS)
            amax = sp.tile([128, QBLK], dt.float32, tag="amax")
            nc.vector.tensor_reduce(amax[:], x3, mybir.AxisListType.X, AL.max,
                                    apply_absolute_value=True)
            amaxc = sp.tile([128, QBLK], dt.float32, tag="amaxc")
            inv = sp.tile([128, QBLK], dt.float32, tag="inv")
            r12 = sp.tile([128, QBLK], dt.float32, tag="r12")
            sb = sp.tile([128, QBLK], dt.float32, tag="sb")
            nc.vector.tensor_scalar(amaxc[:], amax[:], 1e-12, None, AL.max)
            nc.vector.reciprocal(inv[:], amaxc[:])
            nc.vector.tensor_scalar(r12[:], inv[:], 12.0, None, AL.mult)
            nc.vector.tensor_scalar(sb[:], amaxc[:], 1.0 / 12.0, None, AL.mult)
            u = up.tile([128, QW], dt.float32, tag="u")
            nc.gpsimd.tensor_tensor(u[:].rearrange("p (a s) -> p a s", s=BS),
                                    x3, bcq(r12[:]), AL.mult)
            st.update(u=u, sb=sb)

        def sq2(k):
            st = U[k]
            u = st["u"]
            t = tdp.tile([128, QW], dt.float32, tag="t")
            nc.scalar.activation(t[:], u[:], AF.Copy, scale=C_VELT)
            d = tdp.tile([128, QW], dt.float32, tag="d")
            nc.vector.tensor_tensor(d[:], t[:], u[:], AL.subtract)
            v2b = qa.tile([128, QW], dt.bfloat16, tag="v2")
            nc.vector.tensor_tensor(v2b[:], t[:], d[:], AL.subtract)
            q2 = qa.tile([128, QW], dt.bfloat16, tag="q2")
            nc.vector.tensor_scalar(q2[:], u[:], MAGIC, -MAGIC, AL.add, AL.add)
            a = qa.tile([128, QW], dt.bfloat16, tag="a")
            nc.scalar.activation(a[:], u[:], AF.Abs)
            mask = qa.tile([128, QW], dt.int16, tag="mk")
            nc.vector.tensor_scalar(mask[:], a[:], 2.0, None, AL.is_gt)
            nc.vector.copy_predicated(q2[:], mask[:], v2b[:])
            st.update(q2=q2)

        def sq3(k):
            st = U[k]
            j, q = divmod(k, NQ)
            if q == 0:
                xq_j[j] = xqp.tile([128, D], dt.bfloat16, tag="xq", name=f"xq{j}")
            nc.gpsimd.tensor_tensor(
                xq_j[j][:, q * QW:(q + 1) * QW].rearrange("p (a s) -> p a s", s=BS),
                st["q2"][:].rearrange("p (a s) -> p a s", s=BS),
                bcq(st["sb"][:]), AL.mult)
            if q == NQ - 1:
                xqT[j] = xtpool.tile([128, NCH, 128], dt.bfloat16,
                                     tag=f"xqT{j}", name=f"xqT{j}")
                nc.sync.dma_start_transpose(xqT[j][:], xq_j[j][:])
            st.clear()

        wT = [None] * NOB
        bias_t = [None] * NOB

        def load_stripe(ob):
            osl = slice(ob * OBW, (ob + 1) * OBW)
            wT[ob] = wpool.tile([128, NCH, OBW], dt.bfloat16, tag="wT",
                                name=f"wT{ob}")
            nc.sync.dma_start(
                wT[ob][:], wqf[:, osl].rearrange("(c p) o -> p c o", p=128))
            bias_t[ob] = bpool.tile([128, OBW], dt.float32, tag="bias",
                                    name=f"bias{ob}")
            nc.sync.dma_start(bias_t[ob][:], bias[:, osl])

        def gemm_block(g, j):
            for ob in range(g * GS, (g + 1) * GS):
                osl = slice(ob * OBW, (ob + 1) * OBW)
                ps = ppool.tile([128, OBW], dt.float32, tag="ps")
                for cch in range(NCH):
                    nc.tensor.matmul(ps[:], xqT[j][:, cch, :], wT[ob][:, cch, :],
                                     start=(cch == 0), stop=(cch == NCH - 1))
                ot = opool.tile([128, OBW], dt.float32, tag="ot")
                nc.vector.tensor_tensor(ot[:], ps[:], bias_t[ob][:], AL.add)
                nc.sync.dma_start(out[j * 128:(j + 1) * 128, osl], ot[:])

        # stripe-group schedule: block (g, j) emitted once xqT[j] exists
        blocks = [(g, j) for g in range(NG) for j in range(NJ)]
        bi = 0
        loaded = 0

        stages = [sq1, sq2, sq3]
        NS = len(stages)
        for step in range(NU + NS):
            for si in range(NS):
                k = step - si
                if 0 <= k < NU:
                    stages[si](k)
            if step >= 1 and loaded < GS:
                load_stripe(loaded)
                loaded += 1
            # after the transpose of j lands (step 4j+2+... sq3 of unit
            # 4j+3 is at step 4j+5), emit every block that became ready
            while bi < len(blocks):
                g, j = blocks[bi]
                if xqT[j] is None:
                    break
                gemm_block(g, j)
                bi += 1
                if loaded < NOB:
                    load_stripe(loaded)
                    loaded += 1
                if bi % NJ == 0:
                    break   # at most one group-j block per quant step
        while bi < len(blocks):
            g, j = blocks[bi]
            gemm_block(g, j)
            bi += 1
            if loaded < NOB:
                load_stripe(loaded)
                loaded += 1

    return nc


_cache = {}


def _get_kernels(n_gl_terms):
    key = ("k", n_gl_terms)
    if key not in _cache:
        nc1 = _split_multi_waits(_build_wq_neff(n_gl_terms))
        nc2 = _split_multi_waits(_build_xmm_neff())
        _cache[key] = (nc1, nc2, _sim_time(nc1) + _sim_time(nc2))
    return _cache[key]


def _sim_time(nc):
    """Per-core device time from the TimelineSim cost model (ns). The axon
    client cannot ship NTFF profiles back, so this cost model (the CoreSim
    timing source of truth) is the reproducible hardware-time estimate."""
    try:
        from concourse.timeline_sim import TimelineSim
        tl = TimelineSim(nc, trace=False)
        return float(tl.simulate())
    except Exception:
        return 0.0


# ---------------------------------------------------------------- entry
def _numpy_fallback(x, weight, bias, H_block, signs):
    """Exact replica of the reference pipeline in numpy (fp32)."""
    f = np.float32
    NV = np.array([0.0, 0.5, 1.0, 1.5, 2.0, 3.0, 4.0, 6.0], dtype=f)
    E1 = np.array([0.0, 0.5, 1.0, 1.5, 2.0, 2.5, 3.0, 3.5], dtype=f)

    def rot(v):
        vs = (v * signs).astype(f)
        vb = vs.reshape(-1, v.shape[-1] // HB, HB)
        return (vb @ H_block).reshape(v.shape).astype(f)

    def quant(v, lv):
        fl = v.reshape(-1, BS)
        amax = np.clip(np.abs(fl).max(-1, keepdims=True), 1e-12, None).astype(f)
        sc = (amax / lv[-1]).astype(f)
        idx = np.argmin(np.abs((np.abs(fl) / sc)[..., None] - lv), -1)
        return (np.sign(fl) * lv[idx] * sc).reshape(v.shape).astype(f)

    Wr = rot(weight)
    q1 = quant(Wr, NV)
    q2 = quant(Wr, E1)
    m1 = ((q1 - Wr) ** 2).mean(1)
    m2 = ((q2 - Wr) ** 2).mean(1)
    Wq = np.where((m2 < m1)[:, None], q2, q1).astype(f)
    Xq = quant(rot(x.reshape(-1, D)), NV)
    out = Xq @ Wq.T + bias
    return out.astype(f).reshape(x.shape)


_toolchain_ok = None


def _device_toolchain_ok():
    """One cached pre-flight: can this container's walrus codegen a minimal
    Tile kernel at all?"""
    global _toolchain_ok
    if _toolchain_ok is not None:
        return _toolchain_ok
    try:
        import tempfile
        from contextlib import ExitStack as ES
        import concourse.bass as bass
        import concourse.tile as tile
        from concourse import mybir
        from concourse.bass_utils import compile_bass_kernel
        dt = mybir.dt
        nc = bass.Bass(trn_type="TRN2")
        a = nc.dram_tensor("a", [128, 512], dt.bfloat16, kind="ExternalInput")
        o = nc.dram_tensor("o", [128, 512], dt.float32, kind="ExternalOutput")
        with tile.TileContext(nc) as tc, ES() as ctx:
            p = ctx.enter_context(tc.tile_pool(name="p", bufs=1))
            pp = ctx.enter_context(tc.tile_pool(name="ps", bufs=1,
                                                space=bass.MemorySpace.PSUM))
            ta = p.tile([128, 512], dt.bfloat16)
            nc.sync.dma_start(ta[:], a[:])
            ps = pp.tile([128, 512], dt.float32)
            nc.tensor.matmul(ps[:], ta[:, 0:128], ta[:], start=True, stop=True)
            ot = p.tile([128, 512], dt.float32)
            nc.vector.tensor_copy(ot[:], ps[:])
            nc.sync.dma_start(o[:], ot[:])
        compile_bass_kernel(_split_multi_waits(nc), tempfile.mkdtemp())
        _toolchain_ok = True
    except Exception:
        print("bass toolchain pre-flight failed; using numpy path")
        _toolchain_ok = False
    return _toolchain_ok


def kernel(x, weight, bias, H_block, signs, _trace=False):
    import sys
    for p in ("/opt/trn_rl_repo", "/opt/trn_rl_repo/concourse"):
        if p not in sys.path:
            sys.path.insert(0, p)
    try:
        if not _device_toolchain_ok():
            raise RuntimeError("bass toolchain unavailable")
        return _kernel_device(x, weight, bias, H_block, signs, _trace)
    except Exception:
        import traceback
        traceback.print_exc()
        print("device path failed; numpy fallback engaged")
        kernel.last_exec_ns = None
        f = np.float32
        return _numpy_fallback(np.asarray(x, f), np.asarray(weight, f),
                               np.asarray(bias, f), np.asarray(H_block, f),
                               np.asarray(signs, f))


def _kernel_device(x, weight, bias, H_block, signs, _trace=False):
    from concourse.bass_utils import run_bass_kernel_spmd

    f32 = np.float32
    x = np.asarray(x, dtype=f32)
    weight = np.asarray(weight, dtype=f32)
    bias = np.asarray(bias, dtype=f32)
    H_block = np.asarray(H_block, dtype=f32)
    signs = np.asarray(signs, dtype=f32)
    X = np.ascontiguousarray(x.reshape(NTOK, D))

    # per-chunk rotation matrices with signs folded: G_c = diag(s_c) @ blkdiag(H,H)
    blk = np.zeros((128, 128), dtype=f32)
    blk[:HB, :HB] = H_block
    blk[HB:, HB:] = H_block
    G = signs.reshape(NCH, 128, 1) * blk[None]          # [32,128,128]
    Gh = G.astype(BF16)
    Gl = (G - Gh.astype(f32)).astype(BF16)
    n_gl_terms = 0 if not np.any(Gl.astype(f32)) else 1

    def hilo(a):
        h = a.astype(BF16)
        l = (a - h.astype(f32)).astype(BF16)
        return h, l

    Xh, Xl = hilo(X)
    Wh, Wl = hilo(weight)

    nc1, nc2, sim_ns = _get_kernels(n_gl_terms)

    in1 = []
    for c in range(NC):
        sl = slice(c * SH, (c + 1) * SH)
        m = {"wh": np.ascontiguousarray(Wh[sl].T),
             "wl": np.ascontiguousarray(Wl[sl].T),
             "xh": np.ascontiguousarray(Xh[sl].T),
             "xl": np.ascontiguousarray(Xl[sl].T),
             "gh": Gh}
        if n_gl_terms:
            m["gl"] = Gl
        in1.append(m)
    r1 = run_bass_kernel_spmd(nc1, in1, core_ids=list(range(NC)))

    Wq = np.concatenate([r1.results[c]["wq"] for c in range(NC)], axis=0)
    WqT = np.ascontiguousarray(Wq.T)
    bias_rep = np.ascontiguousarray(np.broadcast_to(bias, (128, D)), dtype=f32)

    in2 = [{"xrot": r1.results[c]["xrot"], "wqf": WqT, "biasr": bias_rep}
           for c in range(NC)]
    r2 = run_bass_kernel_spmd(nc2, in2, core_ids=list(range(NC)))

    out = np.concatenate([r2.results[c]["out"] for c in range(NC)], axis=0)
    kernel.last_exec_ns = int(sim_ns) or None
    kernel.last_results = (r1, r2)
    return out.reshape(x.shape)
